# revision 1
# baseline (speedup 1.0000x reference)
"""GCN message-passing kernel (nn_Encoder_21646635172361) for 8 Trainium2 cores.

Math (reference):
    h   = x @ W.T                     [N,H]
    A~  = adjacency + self loops, symmetric-normalized: norm(r,c) = dinv[r]*dinv[c]
    out = PReLU(A~ @ h + b, alpha)

Key algebraic restructure: aggregation commutes with the linear transform,
    A~ @ (x W.T) = (A~ @ x) W.T
so we aggregate F=128-wide rows (4x less gather traffic than H=512).
    agg[c] = dinv[c] * ( sum_{r->c} dinv[r]*x[r] + dinv[c]*x[c] )
Host prescales xs = dinv*x (fp16), so per-edge messages are plain rows of xs
and the scatter-add becomes binary one-hot matmuls on the TensorEngine.
Self-loops are NOT gathered: their xs rows are shipped per-core in local
order (xloc) and folded into each window's matmul chain via two static
shifted-identity masks; the epilogue's dinv[dst] completes dinv^2.

Distribution: destination nodes sharded round-robin (dst % 8) across the 8
cores; xs replicated; each core gathers the source rows for its own edges
(DistGNN-style edge partition, no collectives needed).

Performance notes (measured on HW):
 - dma_gather costs ~9 ns/index on GpSimd regardless of elem size/dtype --
   it is the hard floor of this kernel (~52 calls x 1024 idx ~ 470 us).
 - Everything else (DVE one-hots in fp16, fp16 PE matmuls, scalar-engine
   epilogue) is sized to hide under the gather stream; gxpool is 16 chunks
   deep so the gather never stalls on buffer recycling.
"""

import os
import time
from contextlib import ExitStack

import numpy as np

N, F, H = 50000, 128, 512
NC_CORES = 8
ND = N // NC_CORES            # 6250 local dst nodes per core
WIN = 256                     # dst window width
NW = (ND + WIN - 1) // WIN    # 25 windows
NDP = NW * WIN                # 6400 padded local dst rows
SPLIT = 32768                 # int16 gather index split point
CH = 1024                     # gather chunk size (edges per dma_gather; >1024 fails on HW)
TILE = 128                    # edges per PE tile

# Results of the last kernel() call (for test.py introspection)
last_run_info = {}


def _plan(edge_index, x, W, alpha):
    """Host-side graph partition + input prep. Returns per-core arrays +
    shared tile structure (uniform across cores, required for SPMD)."""
    src = np.asarray(edge_index[0], dtype=np.int64)
    dst = np.asarray(edge_index[1], dtype=np.int64)

    # degrees include self-loops (reference adds them)
    deg = np.bincount(dst, minlength=N) + 1
    dinv = (1.0 / np.sqrt(deg.astype(np.float64))).astype(np.float32)

    # xs rows serve both edge messages (epilogue adds dinv[dst]) and
    # self-loops (same epilogue factor completes dinv[d]^2).
    xs = (dinv[:, None] * x).astype(np.float16)           # [N, F]

    core = (dst % NC_CORES).astype(np.int64)
    loc = dst // NC_CORES
    win = loc // WIN
    dloc = (loc % WIN).astype(np.float32)
    low = src < SPLIT

    # group edges per (core, window, stream)
    grp = {}
    for k in range(NC_CORES):
        mk = core == k
        s_k, w_k, dl_k, lo_k = src[mk], win[mk], dloc[mk], low[mk]
        for w in range(NW):
            mw = w_k == w
            s_w, dl_w, lo_w = s_k[mw], dl_k[mw], lo_k[mw]
            grp[(k, w, 0)] = (s_w[lo_w], dl_w[lo_w])
            grp[(k, w, 1)] = (s_w[~lo_w] - SPLIT, dl_w[~lo_w])

    # uniform tile counts across cores
    T = np.zeros((2, NW), dtype=np.int64)
    for st in range(2):
        for w in range(NW):
            cnt = max(len(grp[(k, w, st)][0]) for k in range(NC_CORES))
            T[st, w] = (cnt + TILE - 1) // TILE
    tile_start = np.zeros((2, NW), dtype=np.int64)
    tile_start[0, 1:] = np.cumsum(T[0])[:-1]
    tile_start[1, 1:] = np.cumsum(T[1])[:-1]
    NT = [int(T[0].sum()), int(T[1].sum())]

    per_core = []
    for k in range(NC_CORES):
        core_dat = {}
        for st in range(2):
            nt = NT[st]
            idx = np.zeros(nt * TILE, dtype=np.int16)      # pad -> row 0 (harmless)
            dlc = np.full(nt * TILE, -1.0, dtype=np.float32)  # pad -> no one-hot match
            for w in range(NW):
                s_w, dl_w = grp[(k, w, st)]
                o = tile_start[st, w] * TILE
                n = len(s_w)
                idx[o:o + n] = s_w.astype(np.int16)
                dlc[o:o + n] = dl_w
            # wrapped int16 index layout: [p, j] = idx[j*16 + p%16], replicated
            wrapped = idx.reshape(-1, 16).T            # [16, nt*8]
            wrapped = np.tile(wrapped, (8, 1)).copy()  # [128, nt*8]
            core_dat[("idx", st)] = wrapped
            core_dat[("dlc", st)] = dlc.reshape(nt, TILE).T.copy()  # [128, nt]
        # local dst rows (xloc) and dinv tables in window-local order
        g = np.arange(NDP, dtype=np.int64) * NC_CORES + k
        valid = np.arange(NDP) < ND
        gc = np.minimum(g, N - 1)
        xl = np.where(valid[:, None], xs[gc], np.float16(0.0))      # [NDP, F]
        core_dat["xloc"] = np.ascontiguousarray(xl)
        dv = np.where(valid, dinv[gc], 0.0).astype(np.float32)
        core_dat["dv"] = dv.reshape(2 * NW, TILE).T.copy()          # [128, 50]
        core_dat["dvn"] = (-core_dat["dv"]).copy()
        per_core.append(core_dat)

    return per_core, T, tile_start, NT, xs


def _build_program(T, tile_start, NT, fast_path):
    import concourse.mybir as mybir
    import concourse.tile as tile
    from concourse import bacc

    f32 = mybir.dt.float32
    fp16 = mybir.dt.float16
    i16 = mybir.dt.int16
    Alu = mybir.AluOpType
    Act = mybir.ActivationFunctionType

    nc = bacc.Bacc("TRN2", target_bir_lowering=False, debug=False,
                   num_devices=NC_CORES)

    xs_d = nc.dram_tensor("xs", [N, F], fp16, kind="ExternalInput").ap()
    xloc_d = nc.dram_tensor("xloc", [NDP, F], fp16, kind="ExternalInput").ap()
    wt_d = nc.dram_tensor("w_t", [F, H], fp16, kind="ExternalInput").ap()
    w1t_d = nc.dram_tensor("w1_t", [F, H], fp16, kind="ExternalInput").ap()
    idx_d = [nc.dram_tensor(f"idx{st}", [128, NT[st] * 8], i16,
                            kind="ExternalInput").ap() for st in range(2)]
    dlc_d = [nc.dram_tensor(f"dlc{st}", [128, NT[st]], f32,
                            kind="ExternalInput").ap() for st in range(2)]
    dv_d = nc.dram_tensor("dv", [128, 2 * NW], f32, kind="ExternalInput").ap()
    dvn_d = nc.dram_tensor("dvn", [128, 2 * NW], f32, kind="ExternalInput").ap()
    dls_d = nc.dram_tensor("dlself", [128, 2], f32, kind="ExternalInput").ap()
    if not fast_path:
        arow_d = nc.dram_tensor("alpha_row", [1, H], f32, kind="ExternalInput").ap()
        brow_d = nc.dram_tensor("b_row", [1, H], f32, kind="ExternalInput").ap()
    out_d = nc.dram_tensor("out", [NDP, H], f32, kind="ExternalOutput").ap()

    xs_lo = xs_d[0:SPLIT, :]
    xs_hi = xs_d[SPLIT:N, :]
    x_in = [xs_lo, xs_hi]

    n_chunks = [(NT[st] * TILE + CH - 1) // CH for st in range(2)]

    with tile.TileContext(nc) as tc, ExitStack() as ctx:
        cpool = ctx.enter_context(tc.tile_pool(name="const", bufs=1))
        gxpool = ctx.enter_context(tc.tile_pool(name="gx", bufs=16))
        xlpool = ctx.enter_context(tc.tile_pool(name="xl", bufs=6))
        ohpool = ctx.enter_context(tc.tile_pool(name="oh", bufs=12))
        aggpool = ctx.enter_context(tc.tile_pool(name="aggs", bufs=4))
        eppool = ctx.enter_context(tc.tile_pool(name="ep", bufs=2))
        ps_agg = ctx.enter_context(tc.tile_pool(name="ps_agg", bufs=2, space="PSUM"))
        ps_out = ctx.enter_context(tc.tile_pool(name="ps_out", bufs=2, space="PSUM"))

        # ---- one-time loads ----
        wt_sb = cpool.tile([F, H], fp16)
        nc.sync.dma_start(wt_sb[:], wt_d)
        if fast_path:
            w1t_sb = cpool.tile([F, H], fp16)
            nc.sync.dma_start(w1t_sb[:], w1t_d)
        idx_sb, dlc_sb = [], []
        for st in range(2):
            t = cpool.tile([128, NT[st] * 8], i16, tag=f"idx{st}")
            nc.sync.dma_start(t[:], idx_d[st])
            idx_sb.append(t)
            t = cpool.tile([128, NT[st]], f32, tag=f"dlc{st}")
            nc.sync.dma_start(t[:], dlc_d[st])
            dlc_sb.append(t)
        dv_sb = cpool.tile([128, 2 * NW], f32)
        nc.sync.dma_start(dv_sb[:], dv_d)
        dvn_sb = cpool.tile([128, 2 * NW], f32)
        nc.sync.dma_start(dvn_sb[:], dvn_d)
        dls_sb = cpool.tile([128, 2], f32)
        nc.sync.dma_start(dls_sb[:], dls_d)

        iota16 = cpool.tile([128, WIN], fp16)
        nc.gpsimd.iota(iota16[:], pattern=[[1, WIN]], base=0,
                       channel_multiplier=0,
                       allow_small_or_imprecise_dtypes=True)

        # static shifted-identity masks for the two self-loop tiles per window
        selfoh = []
        for h in range(2):
            t = cpool.tile([128, WIN], fp16, tag=f"selfoh{h}")
            nc.vector.tensor_scalar(t[:], iota16[:], dls_sb[:, h:h + 1], None,
                                    op0=Alu.is_equal)
            selfoh.append(t)

        if not fast_path:
            ones_sb = cpool.tile([1, 128], f32)
            nc.vector.memset(ones_sb[:], 1.0)
            arow_sb = cpool.tile([1, H], f32)
            nc.sync.dma_start(arow_sb[:], arow_d)
            brow_sb = cpool.tile([1, H], f32)
            nc.sync.dma_start(brow_sb[:], brow_d)
            arep_ps = ps_out.tile([128, H], f32, tag="brd")
            nc.tensor.matmul(arep_ps[:], lhsT=ones_sb[:], rhs=arow_sb[:],
                             start=True, stop=True)
            arep_sb = cpool.tile([128, H], f32)
            nc.scalar.copy(arep_sb[:], arep_ps[:])
            brep_ps = ps_out.tile([128, H], f32, tag="brd")
            nc.tensor.matmul(brep_ps[:], lhsT=ones_sb[:], rhs=brow_sb[:],
                             start=True, stop=True)
            brep_sb = cpool.tile([128, H], f32)
            nc.scalar.copy(brep_sb[:], brep_ps[:])

        # ---- main loop ----
        gx_tiles = [[None] * n_chunks[0], [None] * n_chunks[1]]

        def chunk_tile(st, c):
            if gx_tiles[st][c] is None:
                num = min(CH, NT[st] * TILE - c * CH)
                nblk = num // TILE
                gx = gxpool.tile([128, CH // TILE, TILE], fp16, tag="gx")
                nc.gpsimd.dma_gather(
                    out_ap=gx[:, 0:nblk, :],
                    in_ap=x_in[st],
                    idxs_ap=idx_sb[st][:, c * (CH // 16): c * (CH // 16) + num // 16],
                    num_idxs=num,
                    num_idxs_reg=num,
                    elem_size=F,
                )
                gx_tiles[st][c] = gx
            return gx_tiles[st][c]

        for w in range(NW):
            pagg = ps_agg.tile([128, WIN], f32, tag="pagg")
            n_mm = 2 + int(T[0, w] + T[1, w])
            mm_i = 0
            # self-loop tiles first (independent of the gather stream)
            for h in range(2):
                xl = xlpool.tile([128, F], fp16, tag="xl")
                r0 = (2 * w + h) * 128
                nc.sync.dma_start(xl[:], xloc_d[r0:r0 + 128, :])
                nc.tensor.matmul(pagg[:], lhsT=xl[:], rhs=selfoh[h][:],
                                 start=(mm_i == 0), stop=(mm_i == n_mm - 1))
                mm_i += 1
            for st in range(2):
                for t in range(int(T[st, w])):
                    gt = int(tile_start[st, w]) + t
                    c, blk = divmod(gt, CH // TILE)
                    gx = chunk_tile(st, c)
                    oh = ohpool.tile([128, WIN], fp16, tag="oh")
                    nc.vector.tensor_scalar(
                        oh[:], iota16[:], dlc_sb[st][:, gt:gt + 1], None,
                        op0=Alu.is_equal,
                    )
                    nc.tensor.matmul(
                        pagg[:],
                        lhsT=gx[:, blk:blk + 1, :],
                        rhs=oh[:],
                        start=(mm_i == 0), stop=(mm_i == n_mm - 1),
                    )
                    mm_i += 1
            agg_sb = aggpool.tile([128, WIN], fp16, tag="aggs")
            nc.scalar.copy(agg_sb[:], pagg[:])

            for h2 in range(2):
                hw = 2 * w + h2
                lhs = agg_sb[:, h2 * 128:(h2 + 1) * 128]
                dv_col = dv_sb[:, hw:hw + 1]
                ps0 = ps_out.tile([128, H], f32, tag="ps0")
                nc.tensor.matmul(ps0[:], lhsT=lhs, rhs=wt_sb[:],
                                 start=True, stop=True)
                if fast_path:
                    # out = relu(dv*z0) - relu(-dv*z1), z1 = agg @ (alpha W)^T
                    ps1 = ps_out.tile([128, H], f32, tag="ps1")
                    nc.tensor.matmul(ps1[:], lhsT=lhs, rhs=w1t_sb[:],
                                     start=True, stop=True)
                    pos = eppool.tile([128, H], f32, tag="pos")
                    nc.scalar.activation(pos[:], ps0[:], Act.Relu, scale=dv_col)
                    neg = eppool.tile([128, H], f32, tag="neg")
                    nc.scalar.activation(neg[:], ps1[:], Act.Relu,
                                         scale=dvn_sb[:, hw:hw + 1])
                    outt = eppool.tile([128, H], f32, tag="outt")
                    nc.vector.tensor_tensor(outt[:], pos[:], neg[:],
                                            op=Alu.subtract)
                else:
                    # general: v = dv*z0 + b; out = relu(v) + alpha*min(v,0)
                    vb = eppool.tile([128, H], f32, tag="vb")
                    nc.vector.tensor_scalar(vb[:], ps0[:], dv_col, None,
                                            op0=Alu.mult)
                    vb2 = eppool.tile([128, H], f32, tag="vb2")
                    nc.vector.tensor_tensor(vb2[:], vb[:], brep_sb[:],
                                            op=Alu.add)
                    pos = eppool.tile([128, H], f32, tag="pos")
                    nc.scalar.activation(pos[:], vb2[:], Act.Relu)
                    neg = eppool.tile([128, H], f32, tag="neg")
                    nc.vector.tensor_scalar(neg[:], vb2[:], 0.0, None,
                                            op0=Alu.min)
                    nega = eppool.tile([128, H], f32, tag="nega")
                    nc.vector.tensor_tensor(nega[:], neg[:], arep_sb[:],
                                            op=Alu.mult)
                    outt = eppool.tile([128, H], f32, tag="outt")
                    nc.vector.tensor_tensor(outt[:], pos[:], nega[:],
                                            op=Alu.add)
                nc.sync.dma_start(out_d[hw * 128:(hw + 1) * 128, :], outt[:])

    nc.compile()
    return nc


def kernel(x, edge_index, W, b, alpha):
    from concourse.bass_utils import run_bass_kernel_spmd

    t0 = time.time()
    x = np.ascontiguousarray(np.asarray(x, dtype=np.float32))
    W = np.asarray(W, dtype=np.float32)
    b = np.asarray(b, dtype=np.float32)
    alpha = np.asarray(alpha, dtype=np.float32)

    per_core, T, tile_start, NT, xs = _plan(edge_index, x, W, alpha)
    fast_path = bool(np.all(b == 0.0) and np.all(alpha > 0.0))

    wt = np.ascontiguousarray(W.T.astype(np.float16))                     # [F, H]
    w1t = np.ascontiguousarray((alpha[:, None] * W).T.astype(np.float16))  # [F, H]
    dlself = np.stack([np.arange(128, dtype=np.float32),
                       np.arange(128, dtype=np.float32) + 128], axis=1)
    dlself = np.ascontiguousarray(dlself)  # [128, 2]

    t1 = time.time()
    nc = _build_program(T, tile_start, NT, fast_path)
    t2 = time.time()

    in_maps = []
    for k in range(NC_CORES):
        d = per_core[k]
        m = {
            "xs": xs, "w_t": wt, "w1_t": w1t,
            "xloc": d["xloc"],
            "idx0": d[("idx", 0)], "idx1": d[("idx", 1)],
            "dlc0": d[("dlc", 0)], "dlc1": d[("dlc", 1)],
            "dv": d["dv"], "dvn": d["dvn"],
            "dlself": dlself,
        }
        if not fast_path:
            m["alpha_row"] = alpha.reshape(1, H).astype(np.float32)
            m["b_row"] = b.reshape(1, H).astype(np.float32)
        in_maps.append(m)

    trace = bool(int(os.environ.get("GCN_BASS_TRACE", "0")))
    res = run_bass_kernel_spmd(nc, in_maps, core_ids=list(range(NC_CORES)),
                               trace=trace)
    t3 = time.time()

    outs = np.stack([res.results[k]["out"][:ND] for k in range(NC_CORES)])  # [8, 6250, H]
    out_full = outs.transpose(1, 0, 2).reshape(N, H)
    t4 = time.time()

    last_run_info.update({
        "exec_time_ns": res.exec_time_ns,
        "plan_s": t1 - t0, "build_s": t2 - t1, "run_s": t3 - t2,
        "unshard_s": t4 - t3, "fast_path": fast_path,
        "NT": NT, "trace": trace,
    })
    return out_full



# revision 2
# speedup vs baseline: 1.6172x; 1.6172x over previous
"""GCN message-passing kernel (nn_Encoder_21646635172361) for 8 Trainium2 cores.

Math (reference):
    h   = x @ W.T                     [N,H]
    A~  = adjacency + self loops, symmetric-normalized: norm(r,c) = dinv[r]*dinv[c]
    out = PReLU(A~ @ h + b, alpha)

Key algebraic restructure: aggregation commutes with the linear transform,
    A~ @ (x W.T) = (A~ @ x) W.T
so we aggregate F=128-wide rows (4x less gather traffic than H=512).
    agg[c] = dinv[c] * ( sum_{r->c} dinv[r]*x[r] + dinv[c]*x[c] )
Host prescales xs = dinv*x (fp16), so per-edge messages are plain rows of xs
and the scatter-add becomes binary one-hot matmuls on the TensorEngine.
Self-loops are NOT gathered: their xs rows are shipped per-core in local
order (xloc) and folded into each window's matmul chain via two static
shifted-identity masks; the epilogue's dinv[dst] completes dinv^2.

Distribution: destination nodes sharded round-robin (dst % 8) across the 8
cores; xs replicated; each core gathers the source rows for its own edges
(DistGNN-style edge partition, no collectives needed).

Performance notes (v2, measured on HW):
 - dma_gather desc-gen runs on the Q7 core pair (2q, 2q+1) of its SWDGE
   queue q: ~8.6us per 1024-idx call per queue, but the 4 queues work
   CONCURRENTLY.  Round-robin chunks across all 4 queues -> ~2.15us/call
   effective (~2.1 ns/idx), 4x the single-queue baseline.
 - TRN2 PE accumulation chains into a single PSUM bank stall ~800ns/matmul
   on the bank read-modify-write; interleaving 4 windows' chains across 4
   PSUM banks runs at ~136ns per [128x256] fp16 matmul.  Windows are
   processed in groups of 4 with round-robin emission.
 - Output is written fp16 (half the HBM write traffic); host converts to
   fp32.  absmax-rel error stays ~5e-4, well under the 2e-2 gate.
"""

import os
import time
from contextlib import ExitStack

import numpy as np

N, F, H = 50000, 128, 512
NC_CORES = 8
ND = N // NC_CORES            # 6250 local dst nodes per core
WIN = 256                     # dst window width
NW = (ND + WIN - 1) // WIN    # 25 windows
NDP = NW * WIN                # 6400 padded local dst rows
SPLIT = 32768                 # int16 gather index split point
CH = 1024                     # gather chunk size (edges per dma_gather; >1024 fails on HW)
TILE = 128                    # edges per PE tile
GRP = 4                       # windows interleaved per PSUM-bank group

# Results of the last kernel() call (for test.py introspection)
last_run_info = {}


def _plan(edge_index, x, W, alpha):
    """Host-side graph partition + input prep. Returns per-core arrays +
    shared tile structure (uniform across cores, required for SPMD)."""
    src = np.asarray(edge_index[0], dtype=np.int64)
    dst = np.asarray(edge_index[1], dtype=np.int64)

    # degrees include self-loops (reference adds them)
    deg = np.bincount(dst, minlength=N) + 1
    dinv = (1.0 / np.sqrt(deg.astype(np.float64))).astype(np.float32)

    # xs rows serve both edge messages (epilogue adds dinv[dst]) and
    # self-loops (same epilogue factor completes dinv[d]^2).
    xs = (dinv[:, None] * x).astype(np.float16)           # [N, F]

    core = (dst % NC_CORES).astype(np.int64)
    loc = dst // NC_CORES
    win = loc // WIN
    dloc = (loc % WIN).astype(np.float32)
    low = src < SPLIT

    # group edges per (core, window, stream)
    grp = {}
    for k in range(NC_CORES):
        mk = core == k
        s_k, w_k, dl_k, lo_k = src[mk], win[mk], dloc[mk], low[mk]
        for w in range(NW):
            mw = w_k == w
            s_w, dl_w, lo_w = s_k[mw], dl_k[mw], lo_k[mw]
            grp[(k, w, 0)] = (s_w[lo_w], dl_w[lo_w])
            grp[(k, w, 1)] = (s_w[~lo_w] - SPLIT, dl_w[~lo_w])

    # uniform tile counts across cores
    T = np.zeros((2, NW), dtype=np.int64)
    for st in range(2):
        for w in range(NW):
            cnt = max(len(grp[(k, w, st)][0]) for k in range(NC_CORES))
            T[st, w] = (cnt + TILE - 1) // TILE
    tile_start = np.zeros((2, NW), dtype=np.int64)
    tile_start[0, 1:] = np.cumsum(T[0])[:-1]
    tile_start[1, 1:] = np.cumsum(T[1])[:-1]
    NT = [int(T[0].sum()), int(T[1].sum())]

    per_core = []
    for k in range(NC_CORES):
        core_dat = {}
        for st in range(2):
            nt = NT[st]
            idx = np.zeros(nt * TILE, dtype=np.int16)      # pad -> row 0 (harmless)
            dlc = np.full(nt * TILE, -1.0, dtype=np.float32)  # pad -> no one-hot match
            for w in range(NW):
                s_w, dl_w = grp[(k, w, st)]
                o = tile_start[st, w] * TILE
                n = len(s_w)
                idx[o:o + n] = s_w.astype(np.int16)
                dlc[o:o + n] = dl_w
            # wrapped int16 index layout: [p, j] = idx[j*16 + p%16], replicated
            wrapped = idx.reshape(-1, 16).T            # [16, nt*8]
            wrapped = np.tile(wrapped, (8, 1)).copy()  # [128, nt*8]
            core_dat[("idx", st)] = wrapped
            core_dat[("dlc", st)] = dlc.reshape(nt, TILE).T.copy()  # [128, nt]
        # local dst rows (xloc) and dinv tables in window-local order
        g = np.arange(NDP, dtype=np.int64) * NC_CORES + k
        valid = np.arange(NDP) < ND
        gc = np.minimum(g, N - 1)
        xl = np.where(valid[:, None], xs[gc], np.float16(0.0))      # [NDP, F]
        core_dat["xloc"] = np.ascontiguousarray(xl)
        dv = np.where(valid, dinv[gc], 0.0).astype(np.float32)
        core_dat["dv"] = dv.reshape(2 * NW, TILE).T.copy()          # [128, 50]
        core_dat["dvn"] = (-core_dat["dv"]).copy()
        per_core.append(core_dat)

    return per_core, T, tile_start, NT, xs


def _build_program(T, tile_start, NT, fast_path):
    import concourse.mybir as mybir
    import concourse.tile as tile
    from concourse import bacc

    f32 = mybir.dt.float32
    fp16 = mybir.dt.float16
    i16 = mybir.dt.int16
    Alu = mybir.AluOpType
    Act = mybir.ActivationFunctionType

    nc = bacc.Bacc("TRN2", target_bir_lowering=False, debug=False,
                   num_devices=NC_CORES, num_swdge_queues=4)

    xs_d = nc.dram_tensor("xs", [N, F], fp16, kind="ExternalInput").ap()
    xloc_d = nc.dram_tensor("xloc", [NDP, F], fp16, kind="ExternalInput").ap()
    wt_d = nc.dram_tensor("w_t", [F, H], fp16, kind="ExternalInput").ap()
    w1t_d = nc.dram_tensor("w1_t", [F, H], fp16, kind="ExternalInput").ap()
    idx_d = [nc.dram_tensor(f"idx{st}", [128, NT[st] * 8], i16,
                            kind="ExternalInput").ap() for st in range(2)]
    dlc_d = [nc.dram_tensor(f"dlc{st}", [128, NT[st]], f32,
                            kind="ExternalInput").ap() for st in range(2)]
    dv_d = nc.dram_tensor("dv", [128, 2 * NW], f32, kind="ExternalInput").ap()
    dvn_d = nc.dram_tensor("dvn", [128, 2 * NW], f32, kind="ExternalInput").ap()
    dls_d = nc.dram_tensor("dlself", [128, 2], f32, kind="ExternalInput").ap()
    if not fast_path:
        arow_d = nc.dram_tensor("alpha_row", [1, H], f32, kind="ExternalInput").ap()
        brow_d = nc.dram_tensor("b_row", [1, H], f32, kind="ExternalInput").ap()
    out_d = nc.dram_tensor("out", [NDP, H], fp16, kind="ExternalOutput").ap()

    xs_lo = xs_d[0:SPLIT, :]
    xs_hi = xs_d[SPLIT:N, :]
    x_in = [xs_lo, xs_hi]

    n_chunks = [(NT[st] * TILE + CH - 1) // CH for st in range(2)]

    with tile.TileContext(nc) as tc, ExitStack() as ctx:
        cpool = ctx.enter_context(tc.tile_pool(name="const", bufs=1))
        gxpool = ctx.enter_context(tc.tile_pool(name="gx", bufs=16))
        xlpool = ctx.enter_context(tc.tile_pool(name="xl", bufs=10))
        ohpool = ctx.enter_context(tc.tile_pool(name="oh", bufs=16))
        aggpool = ctx.enter_context(tc.tile_pool(name="aggs", bufs=2))
        eppool = ctx.enter_context(tc.tile_pool(name="ep", bufs=2))
        ps_agg = ctx.enter_context(tc.tile_pool(name="ps_agg", bufs=1, space="PSUM"))
        ps_out = ctx.enter_context(tc.tile_pool(name="ps_out", bufs=2, space="PSUM"))

        # ---- one-time loads ----
        wt_sb = cpool.tile([F, H], fp16)
        nc.sync.dma_start(wt_sb[:], wt_d)
        if fast_path:
            w1t_sb = cpool.tile([F, H], fp16)
            nc.sync.dma_start(w1t_sb[:], w1t_d)
        idx_sb, dlc_sb = [], []
        for st in range(2):
            t = cpool.tile([128, NT[st] * 8], i16, tag=f"idx{st}")
            nc.sync.dma_start(t[:], idx_d[st])
            idx_sb.append(t)
            t = cpool.tile([128, NT[st]], f32, tag=f"dlc{st}")
            nc.sync.dma_start(t[:], dlc_d[st])
            dlc_sb.append(t)
        dv_sb = cpool.tile([128, 2 * NW], f32)
        nc.sync.dma_start(dv_sb[:], dv_d)
        dvn_sb = cpool.tile([128, 2 * NW], f32)
        nc.sync.dma_start(dvn_sb[:], dvn_d)
        dls_sb = cpool.tile([128, 2], f32)
        nc.sync.dma_start(dls_sb[:], dls_d)

        iota16 = cpool.tile([128, WIN], fp16)
        nc.gpsimd.iota(iota16[:], pattern=[[1, WIN]], base=0,
                       channel_multiplier=0,
                       allow_small_or_imprecise_dtypes=True)

        # static shifted-identity masks for the two self-loop tiles per window
        selfoh = []
        for h in range(2):
            t = cpool.tile([128, WIN], fp16, tag=f"selfoh{h}")
            nc.vector.tensor_scalar(t[:], iota16[:], dls_sb[:, h:h + 1], None,
                                    op0=Alu.is_equal)
            selfoh.append(t)

        if not fast_path:
            ones_sb = cpool.tile([1, 128], f32)
            nc.vector.memset(ones_sb[:], 1.0)
            arow_sb = cpool.tile([1, H], f32)
            nc.sync.dma_start(arow_sb[:], arow_d)
            brow_sb = cpool.tile([1, H], f32)
            nc.sync.dma_start(brow_sb[:], brow_d)
            arep_ps = ps_out.tile([128, H], f32, tag="brd")
            nc.tensor.matmul(arep_ps[:], lhsT=ones_sb[:], rhs=arow_sb[:],
                             start=True, stop=True)
            arep_sb = cpool.tile([128, H], f32)
            nc.scalar.copy(arep_sb[:], arep_ps[:])
            brep_ps = ps_out.tile([128, H], f32, tag="brd")
            nc.tensor.matmul(brep_ps[:], lhsT=ones_sb[:], rhs=brow_sb[:],
                             start=True, stop=True)
            brep_sb = cpool.tile([128, H], f32)
            nc.scalar.copy(brep_sb[:], brep_ps[:])

        # ---- main loop ----
        gx_tiles = [[None] * n_chunks[0], [None] * n_chunks[1]]
        q_counter = [0]

        def chunk_tile(st, c):
            if gx_tiles[st][c] is None:
                num = min(CH, NT[st] * TILE - c * CH)
                nblk = num // TILE
                gx = gxpool.tile([128, CH // TILE, TILE], fp16, tag="gx")
                nc.gpsimd.dma_gather(
                    out_ap=gx[:, 0:nblk, :],
                    in_ap=x_in[st],
                    idxs_ap=idx_sb[st][:, c * (CH // 16): c * (CH // 16) + num // 16],
                    num_idxs=num,
                    num_idxs_reg=num,
                    elem_size=F,
                    queue_num=q_counter[0] % 4,
                )
                q_counter[0] += 1
                gx_tiles[st][c] = gx
            return gx_tiles[st][c]

        def epilogue(w, pagg):
            agg_sb = aggpool.tile([128, WIN], fp16, tag="aggs")
            nc.scalar.copy(agg_sb[:], pagg[:])
            for h2 in range(2):
                hw = 2 * w + h2
                lhs = agg_sb[:, h2 * 128:(h2 + 1) * 128]
                dv_col = dv_sb[:, hw:hw + 1]
                ps0 = ps_out.tile([128, H], f32, tag="ps0")
                nc.tensor.matmul(ps0[:], lhsT=lhs, rhs=wt_sb[:],
                                 start=True, stop=True)
                if fast_path:
                    # out = relu(dv*z0) - relu(-dv*z1), z1 = agg @ (alpha W)^T
                    ps1 = ps_out.tile([128, H], f32, tag="ps1")
                    nc.tensor.matmul(ps1[:], lhsT=lhs, rhs=w1t_sb[:],
                                     start=True, stop=True)
                    pos = eppool.tile([128, H], f32, tag="pos")
                    nc.scalar.activation(pos[:], ps0[:], Act.Relu, scale=dv_col)
                    neg = eppool.tile([128, H], f32, tag="neg")
                    nc.scalar.activation(neg[:], ps1[:], Act.Relu,
                                         scale=dvn_sb[:, hw:hw + 1])
                    outt = eppool.tile([128, H], fp16, tag="outt")
                    nc.vector.tensor_tensor(outt[:], pos[:], neg[:],
                                            op=Alu.subtract)
                else:
                    # general: v = dv*z0 + b; out = relu(v) + alpha*min(v,0)
                    vb = eppool.tile([128, H], f32, tag="vb")
                    nc.vector.tensor_scalar(vb[:], ps0[:], dv_col, None,
                                            op0=Alu.mult)
                    vb2 = eppool.tile([128, H], f32, tag="vb2")
                    nc.vector.tensor_tensor(vb2[:], vb[:], brep_sb[:],
                                            op=Alu.add)
                    pos = eppool.tile([128, H], f32, tag="pos")
                    nc.scalar.activation(pos[:], vb2[:], Act.Relu)
                    neg = eppool.tile([128, H], f32, tag="neg")
                    nc.vector.tensor_scalar(neg[:], vb2[:], 0.0, None,
                                            op0=Alu.min)
                    nega = eppool.tile([128, H], f32, tag="nega")
                    nc.vector.tensor_tensor(nega[:], neg[:], arep_sb[:],
                                            op=Alu.mult)
                    outt = eppool.tile([128, H], fp16, tag="outt")
                    nc.vector.tensor_tensor(outt[:], pos[:], nega[:],
                                            op=Alu.add)
                nc.sync.dma_start(out_d[hw * 128:(hw + 1) * 128, :], outt[:])

        # windows in groups of GRP; interleave matmul emission across the
        # group so consecutive PE instructions hit different PSUM banks.
        for g0 in range(0, NW, GRP):
            wins = list(range(g0, min(g0 + GRP, NW)))
            paggs = {}
            mm_count = {}
            n_mm = {}
            for j, w in enumerate(wins):
                paggs[w] = ps_agg.tile([128, WIN], f32, name=f"pagg{j}",
                                       tag=f"pagg{j}")
                mm_count[w] = 0
                n_mm[w] = 2 + int(T[0, w] + T[1, w])

            # self-loop matmuls first (start each window's chain)
            for h in range(2):
                for w in wins:
                    xl = xlpool.tile([128, F], fp16, tag="xl")
                    r0 = (2 * w + h) * 128
                    nc.sync.dma_start(xl[:], xloc_d[r0:r0 + 128, :])
                    nc.tensor.matmul(paggs[w][:], lhsT=xl[:], rhs=selfoh[h][:],
                                     start=(mm_count[w] == 0),
                                     stop=(mm_count[w] == n_mm[w] - 1))
                    mm_count[w] += 1

            # round-robin the gather tiles across the group's windows
            tl = {w: [(st, t) for st in range(2) for t in range(int(T[st, w]))]
                  for w in wins}
            pos_i = {w: 0 for w in wins}
            remaining = sum(len(v) for v in tl.values())
            while remaining:
                for w in wins:
                    if pos_i[w] >= len(tl[w]):
                        continue
                    st, t = tl[w][pos_i[w]]
                    pos_i[w] += 1
                    remaining -= 1
                    gt = int(tile_start[st, w]) + t
                    c, blk = divmod(gt, CH // TILE)
                    gx = chunk_tile(st, c)
                    oh = ohpool.tile([128, WIN], fp16, tag="oh")
                    nc.vector.tensor_scalar(
                        oh[:], iota16[:], dlc_sb[st][:, gt:gt + 1], None,
                        op0=Alu.is_equal,
                    )
                    nc.tensor.matmul(
                        paggs[w][:],
                        lhsT=gx[:, blk:blk + 1, :],
                        rhs=oh[:],
                        start=(mm_count[w] == 0),
                        stop=(mm_count[w] == n_mm[w] - 1),
                    )
                    mm_count[w] += 1

            for w in wins:
                epilogue(w, paggs[w])

    nc.compile()
    return nc


def kernel(x, edge_index, W, b, alpha):
    from concourse.bass_utils import run_bass_kernel_spmd

    t0 = time.time()
    x = np.ascontiguousarray(np.asarray(x, dtype=np.float32))
    W = np.asarray(W, dtype=np.float32)
    b = np.asarray(b, dtype=np.float32)
    alpha = np.asarray(alpha, dtype=np.float32)

    per_core, T, tile_start, NT, xs = _plan(edge_index, x, W, alpha)
    fast_path = bool(np.all(b == 0.0) and np.all(alpha > 0.0))

    wt = np.ascontiguousarray(W.T.astype(np.float16))                     # [F, H]
    w1t = np.ascontiguousarray((alpha[:, None] * W).T.astype(np.float16))  # [F, H]
    dlself = np.stack([np.arange(128, dtype=np.float32),
                       np.arange(128, dtype=np.float32) + 128], axis=1)
    dlself = np.ascontiguousarray(dlself)  # [128, 2]

    t1 = time.time()
    nc = _build_program(T, tile_start, NT, fast_path)
    t2 = time.time()

    in_maps = []
    for k in range(NC_CORES):
        d = per_core[k]
        m = {
            "xs": xs, "w_t": wt, "w1_t": w1t,
            "xloc": d["xloc"],
            "idx0": d[("idx", 0)], "idx1": d[("idx", 1)],
            "dlc0": d[("dlc", 0)], "dlc1": d[("dlc", 1)],
            "dv": d["dv"], "dvn": d["dvn"],
            "dlself": dlself,
        }
        if not fast_path:
            m["alpha_row"] = alpha.reshape(1, H).astype(np.float32)
            m["b_row"] = b.reshape(1, H).astype(np.float32)
        in_maps.append(m)

    trace = bool(int(os.environ.get("GCN_BASS_TRACE", "0")))
    res = run_bass_kernel_spmd(nc, in_maps, core_ids=list(range(NC_CORES)),
                               trace=trace)
    t3 = time.time()

    outs = np.stack([res.results[k]["out"][:ND].astype(np.float32)
                     for k in range(NC_CORES)])  # [8, 6250, H]
    out_full = outs.transpose(1, 0, 2).reshape(N, H)
    t4 = time.time()

    last_run_info.update({
        "exec_time_ns": res.exec_time_ns,
        "plan_s": t1 - t0, "build_s": t2 - t1, "run_s": t3 - t2,
        "unshard_s": t4 - t3, "fast_path": fast_path,
        "NT": NT, "trace": trace,
    })
    return out_full


# revision 9
# speedup vs baseline: 2.3102x; 1.4285x over previous
"""GCN message-passing kernel (nn_Encoder_21646635172361) for 8 Trainium2 cores.

Math (reference):
    h   = x @ W.T                     [N,H]
    A~  = adjacency + self loops, symmetric-normalized: norm(r,c) = dinv[r]*dinv[c]
    out = PReLU(A~ @ h + b, alpha)

Key algebraic restructure: aggregation commutes with the linear transform,
    A~ @ (x W.T) = (A~ @ x) W.T
so we aggregate F=128-wide rows (4x less gather traffic than H=512).
    agg[c] = dinv[c] * ( sum_{r->c} dinv[r]*x[r] + dinv[c]*x[c] )
Host prescales xs = dinv*x (fp16), so per-edge messages are plain rows of xs
and the scatter-add becomes binary one-hot matmuls on the TensorEngine.
Self-loops are NOT gathered: their xs rows are shipped per-core in local
order (xloc) and folded into each window's matmul chain via two static
shifted-identity masks; the epilogue's dinv[dst] completes dinv^2.

Distribution: destination nodes sharded round-robin (dst % 8) across the 8
cores; xs replicated; each core gathers the source rows for its own edges
(DistGNN-style edge partition, no collectives needed).

Performance notes (v2, measured on HW):
 - dma_gather desc-gen runs on the Q7 core pair (2q, 2q+1) of its SWDGE
   queue q: ~8.6us per 1024-idx call per queue, but the 4 queues work
   CONCURRENTLY.  Round-robin chunks across all 4 queues -> ~2.15us/call
   effective (~2.1 ns/idx), 4x the single-queue baseline.
 - TRN2 PE accumulation chains into a single PSUM bank stall ~800ns/matmul
   on the bank read-modify-write; interleaving 4 windows' chains across 4
   PSUM banks runs at ~136ns per [128x256] fp16 matmul.  Windows are
   processed in groups of 4 with round-robin emission.
 - Output is written fp16 (half the HBM write traffic); host converts to
   fp32.  absmax-rel error stays ~5e-4, well under the 2e-2 gate.
"""

import os
import time
from contextlib import ExitStack

import numpy as np

N, F, H = 50000, 128, 512
NC_CORES = 8
ND = N // NC_CORES            # 6250 local dst nodes per core
WIN = 256                     # dst window width
NW = (ND + WIN - 1) // WIN    # 25 windows
NDP = NW * WIN                # 6400 padded local dst rows
SPLIT = 32768                 # int16 gather index split point
CH = 1024                     # gather chunk size (edges per dma_gather; >1024 fails on HW)
TILE = 128                    # edges per PE tile
GRP = 4                       # windows interleaved per PSUM-bank group

# Results of the last kernel() call (for test.py introspection)
last_run_info = {}


def _plan(edge_index, x, W, alpha):
    """Host-side graph partition + input prep. Returns per-core arrays +
    shared tile structure (uniform across cores, required for SPMD)."""
    src = np.asarray(edge_index[0], dtype=np.int64)
    dst = np.asarray(edge_index[1], dtype=np.int64)

    # degrees include self-loops (reference adds them)
    deg = np.bincount(dst, minlength=N) + 1
    dinv = (1.0 / np.sqrt(deg.astype(np.float64))).astype(np.float32)

    # xs rows serve both edge messages (epilogue adds dinv[dst]) and
    # self-loops (same epilogue factor completes dinv[d]^2).
    xs = (dinv[:, None] * x).astype(np.float16)           # [N, F]

    core = (dst % NC_CORES).astype(np.int64)
    loc = dst // NC_CORES
    win = loc // WIN
    dloc = (loc % WIN).astype(np.float32)
    low = src < SPLIT

    # group edges per (core, window, stream)
    grp = {}
    for k in range(NC_CORES):
        mk = core == k
        s_k, w_k, dl_k, lo_k = src[mk], win[mk], dloc[mk], low[mk]
        for w in range(NW):
            mw = w_k == w
            s_w, dl_w, lo_w = s_k[mw], dl_k[mw], lo_k[mw]
            grp[(k, w, 0)] = (s_w[lo_w], dl_w[lo_w])
            grp[(k, w, 1)] = (s_w[~lo_w] - SPLIT, dl_w[~lo_w])

    # uniform tile counts across cores
    T = np.zeros((2, NW), dtype=np.int64)
    for st in range(2):
        for w in range(NW):
            cnt = max(len(grp[(k, w, st)][0]) for k in range(NC_CORES))
            T[st, w] = (cnt + TILE - 1) // TILE
    tile_start = np.zeros((2, NW), dtype=np.int64)
    tile_start[0, 1:] = np.cumsum(T[0])[:-1]
    tile_start[1, 1:] = np.cumsum(T[1])[:-1]
    NT = [int(T[0].sum()), int(T[1].sum())]

    per_core = []
    for k in range(NC_CORES):
        core_dat = {}
        for st in range(2):
            nt = NT[st]
            idx = np.zeros(nt * TILE, dtype=np.int16)      # pad -> row 0 (harmless)
            dlc = np.full(nt * TILE, -1.0, dtype=np.float32)  # pad -> no one-hot match
            for w in range(NW):
                s_w, dl_w = grp[(k, w, st)]
                o = tile_start[st, w] * TILE
                n = len(s_w)
                idx[o:o + n] = s_w.astype(np.int16)
                dlc[o:o + n] = dl_w
            # wrapped int16 index layout: [p, j] = idx[j*16 + p%16], replicated
            wrapped = idx.reshape(-1, 16).T            # [16, nt*8]
            wrapped = np.tile(wrapped, (8, 1)).copy()  # [128, nt*8]
            core_dat[("idx", st)] = wrapped
            core_dat[("dlc", st)] = dlc.reshape(nt, TILE).T.copy()  # [128, nt]
        # local dst rows (xloc) and dinv tables in window-local order
        g = np.arange(NDP, dtype=np.int64) * NC_CORES + k
        valid = np.arange(NDP) < ND
        gc = np.minimum(g, N - 1)
        xl = np.where(valid[:, None], xs[gc], np.float16(0.0))      # [NDP, F]
        core_dat["xloc"] = np.ascontiguousarray(xl)
        dv = np.where(valid, dinv[gc], 0.0).astype(np.float32)
        core_dat["dv"] = dv.reshape(2 * NW, TILE).T.copy()          # [128, 50]
        core_dat["dvn"] = (-core_dat["dv"]).copy()
        per_core.append(core_dat)

    return per_core, T, tile_start, NT, xs


def _build_program(T, tile_start, NT, fast_path, uniform_alpha, alpha0):
    import concourse.mybir as mybir
    import concourse.tile as tile
    from concourse import bacc

    f32 = mybir.dt.float32
    fp16 = mybir.dt.float16
    i16 = mybir.dt.int16
    Alu = mybir.AluOpType
    Act = mybir.ActivationFunctionType

    nc = bacc.Bacc("TRN2", target_bir_lowering=False, debug=False,
                   num_devices=NC_CORES, num_swdge_queues=4)

    xs_d = nc.dram_tensor("xs", [N, F], fp16, kind="ExternalInput").ap()
    xloc_d = nc.dram_tensor("xloc", [NDP, F], fp16, kind="ExternalInput").ap()
    wt_d = nc.dram_tensor("w_t", [F, H], fp16, kind="ExternalInput").ap()
    w1t_d = nc.dram_tensor("w1_t", [F, H], fp16, kind="ExternalInput").ap()
    idx_d = [nc.dram_tensor(f"idx{st}", [128, NT[st] * 8], i16,
                            kind="ExternalInput").ap() for st in range(2)]
    dlc_d = [nc.dram_tensor(f"dlc{st}", [128, NT[st]], f32,
                            kind="ExternalInput").ap() for st in range(2)]
    dv_d = nc.dram_tensor("dv", [128, 2 * NW], f32, kind="ExternalInput").ap()
    dvn_d = nc.dram_tensor("dvn", [128, 2 * NW], f32, kind="ExternalInput").ap()
    dls_d = nc.dram_tensor("dlself", [128, 2], f32, kind="ExternalInput").ap()
    if not fast_path:
        arow_d = nc.dram_tensor("alpha_row", [1, H], f32, kind="ExternalInput").ap()
        brow_d = nc.dram_tensor("b_row", [1, H], f32, kind="ExternalInput").ap()
    out_d = nc.dram_tensor("out", [NDP, H], fp16, kind="ExternalOutput").ap()

    xs_lo = xs_d[0:SPLIT, :]
    xs_hi = xs_d[SPLIT:N, :]
    x_in = [xs_lo, xs_hi]

    n_chunks = [(NT[st] * TILE + CH - 1) // CH for st in range(2)]

    n_chunks_tot = n_chunks[0] + n_chunks[1]

    with tile.TileContext(nc) as tc, ExitStack() as ctx:
        cpool = ctx.enter_context(tc.tile_pool(name="const", bufs=1))
        # all gather chunks stay resident: no recycle stalls on the gather
        gxpool = ctx.enter_context(tc.tile_pool(name="gx", bufs=n_chunks_tot))
        xlpool = ctx.enter_context(tc.tile_pool(name="xl", bufs=10))
        ohpool = ctx.enter_context(tc.tile_pool(name="oh", bufs=10))
        aggpool = ctx.enter_context(tc.tile_pool(name="aggs", bufs=2))
        eppool = ctx.enter_context(tc.tile_pool(name="ep", bufs=2))
        ps_agg = ctx.enter_context(tc.tile_pool(name="ps_agg", bufs=1, space="PSUM"))
        ps_out = ctx.enter_context(tc.tile_pool(name="ps_out", bufs=2, space="PSUM"))

        # ---- one-time loads (idx tables first: the gather stream waits on them) ----
        idx_sb, dlc_sb = [], []
        for st in range(2):
            t = cpool.tile([128, NT[st] * 8], i16, tag=f"idx{st}")
            nc.sync.dma_start(t[:], idx_d[st])
            idx_sb.append(t)
        for st in range(2):
            t = cpool.tile([128, NT[st]], f32, tag=f"dlc{st}")
            nc.sync.dma_start(t[:], dlc_d[st])
            dlc_sb.append(t)
        dv_sb = cpool.tile([128, 2 * NW], f32)
        nc.sync.dma_start(dv_sb[:], dv_d)
        dvn_sb = cpool.tile([128, 2 * NW], f32)
        nc.sync.dma_start(dvn_sb[:], dvn_d)
        dls_sb = cpool.tile([128, 2], f32)
        nc.sync.dma_start(dls_sb[:], dls_d)
        wt_sb = cpool.tile([F, H], fp16)
        nc.sync.dma_start(wt_sb[:], wt_d)
        if fast_path and not uniform_alpha:
            w1t_sb = cpool.tile([F, H], fp16)
            nc.sync.dma_start(w1t_sb[:], w1t_d)

        iota16 = cpool.tile([128, WIN], fp16)
        nc.gpsimd.iota(iota16[:], pattern=[[1, WIN]], base=0,
                       channel_multiplier=0,
                       allow_small_or_imprecise_dtypes=True)

        # static shifted-identity masks for the two self-loop tiles per window
        selfoh = []
        for h in range(2):
            t = cpool.tile([128, WIN], fp16, tag=f"selfoh{h}")
            nc.vector.tensor_scalar(t[:], iota16[:], dls_sb[:, h:h + 1], None,
                                    op0=Alu.is_equal)
            selfoh.append(t)

        if not fast_path:
            ones_sb = cpool.tile([1, 128], f32)
            nc.vector.memset(ones_sb[:], 1.0)
            arow_sb = cpool.tile([1, H], f32)
            nc.sync.dma_start(arow_sb[:], arow_d)
            brow_sb = cpool.tile([1, H], f32)
            nc.sync.dma_start(brow_sb[:], brow_d)
            arep_ps = ps_out.tile([128, H], f32, tag="brd")
            nc.tensor.matmul(arep_ps[:], lhsT=ones_sb[:], rhs=arow_sb[:],
                             start=True, stop=True)
            arep_sb = cpool.tile([128, H], f32)
            nc.scalar.copy(arep_sb[:], arep_ps[:])
            brep_ps = ps_out.tile([128, H], f32, tag="brd")
            nc.tensor.matmul(brep_ps[:], lhsT=ones_sb[:], rhs=brow_sb[:],
                             start=True, stop=True)
            brep_sb = cpool.tile([128, H], f32)
            nc.scalar.copy(brep_sb[:], brep_ps[:])

        # ---- main loop ----
        gx_tiles = [[None] * n_chunks[0], [None] * n_chunks[1]]
        oh_tiles = [[None] * n_chunks[0], [None] * n_chunks[1]]
        q_counter = [0]

        def chunk_tile(st, c):
            if gx_tiles[st][c] is None:
                num = min(CH, NT[st] * TILE - c * CH)
                nblk = num // TILE
                gx = gxpool.tile([128, CH // TILE, TILE], fp16, tag="gx")
                nc.gpsimd.dma_gather(
                    out_ap=gx[:, 0:nblk, :],
                    in_ap=x_in[st],
                    idxs_ap=idx_sb[st][:, c * (CH // 16): c * (CH // 16) + num // 16],
                    num_idxs=num,
                    num_idxs_reg=num,
                    elem_size=F,
                    queue_num=q_counter[0] % 4,
                )
                q_counter[0] += 1
                gx_tiles[st][c] = gx
            return gx_tiles[st][c]

        def oh_chunk_tile(st, c):
            # one is_equal per chunk: iota row broadcast over the 8 tile
            # slots, dlc column broadcast over the 256 window positions.
            if oh_tiles[st][c] is None:
                nblk = min(CH // TILE, NT[st] - c * (CH // TILE))
                oh = ohpool.tile([128, CH // TILE, WIN], fp16, tag="oh")
                io_b = iota16[:].unsqueeze(1).to_broadcast([128, nblk, WIN])
                dlc_b = (dlc_sb[st][:, c * (CH // TILE): c * (CH // TILE) + nblk]
                         .unsqueeze(2).to_broadcast([128, nblk, WIN]))
                nc.vector.tensor_tensor(oh[:, 0:nblk, :], io_b, dlc_b,
                                        op=Alu.is_equal)
                oh_tiles[st][c] = oh
            return oh_tiles[st][c]

        def epilogue(w, pagg):
            agg_sb = aggpool.tile([128, WIN], fp16, tag="aggs")
            nc.scalar.copy(agg_sb[:], pagg[:])
            for h2 in range(2):
                hw = 2 * w + h2
                lhs = agg_sb[:, h2 * 128:(h2 + 1) * 128]
                dv_col = dv_sb[:, hw:hw + 1]
                ps0 = ps_out.tile([128, H], f32, tag="ps0")
                nc.tensor.matmul(ps0[:], lhsT=lhs, rhs=wt_sb[:],
                                 start=True, stop=True)
                if uniform_alpha:
                    # out = PReLU(dv*z0; alpha0): single activation, no z1
                    outt = eppool.tile([128, H], fp16, tag="outt")
                    nc.scalar.activation(outt[:], ps0[:], Act.Prelu,
                                         scale=dv_col, alpha=float(alpha0))
                elif fast_path:
                    # out = relu(dv*z0) - relu(-dv*z1), z1 = agg @ (alpha W)^T
                    ps1 = ps_out.tile([128, H], f32, tag="ps1")
                    nc.tensor.matmul(ps1[:], lhsT=lhs, rhs=w1t_sb[:],
                                     start=True, stop=True)
                    pos = eppool.tile([128, H], f32, tag="pos")
                    nc.scalar.activation(pos[:], ps0[:], Act.Relu, scale=dv_col)
                    neg = eppool.tile([128, H], f32, tag="neg")
                    nc.scalar.activation(neg[:], ps1[:], Act.Relu,
                                         scale=dvn_sb[:, hw:hw + 1])
                    outt = eppool.tile([128, H], fp16, tag="outt")
                    nc.vector.tensor_tensor(outt[:], pos[:], neg[:],
                                            op=Alu.subtract)
                else:
                    # general: v = dv*z0 + b; out = relu(v) + alpha*min(v,0)
                    vb = eppool.tile([128, H], f32, tag="vb")
                    nc.vector.tensor_scalar(vb[:], ps0[:], dv_col, None,
                                            op0=Alu.mult)
                    vb2 = eppool.tile([128, H], f32, tag="vb2")
                    nc.vector.tensor_tensor(vb2[:], vb[:], brep_sb[:],
                                            op=Alu.add)
                    pos = eppool.tile([128, H], f32, tag="pos")
                    nc.scalar.activation(pos[:], vb2[:], Act.Relu)
                    neg = eppool.tile([128, H], f32, tag="neg")
                    nc.vector.tensor_scalar(neg[:], vb2[:], 0.0, None,
                                            op0=Alu.min)
                    nega = eppool.tile([128, H], f32, tag="nega")
                    nc.vector.tensor_tensor(nega[:], neg[:], arep_sb[:],
                                            op=Alu.mult)
                    outt = eppool.tile([128, H], fp16, tag="outt")
                    nc.vector.tensor_tensor(outt[:], pos[:], nega[:],
                                            op=Alu.add)
                nc.sync.dma_start(out_d[hw * 128:(hw + 1) * 128, :], outt[:])

        # windows in groups of GRP; interleave matmul emission across the
        # group so consecutive PE instructions hit different PSUM banks.
        for g0 in range(0, NW, GRP):
            wins = list(range(g0, min(g0 + GRP, NW)))
            paggs = {}
            mm_count = {}
            n_mm = {}
            for j, w in enumerate(wins):
                paggs[w] = ps_agg.tile([128, WIN], f32, name=f"pagg{j}",
                                       tag=f"pagg{j}")
                mm_count[w] = 0
                n_mm[w] = 2 + int(T[0, w] + T[1, w])

            # self-loop matmuls first (start each window's chain)
            for h in range(2):
                for w in wins:
                    xl = xlpool.tile([128, F], fp16, tag="xl")
                    r0 = (2 * w + h) * 128
                    nc.sync.dma_start(xl[:], xloc_d[r0:r0 + 128, :])
                    nc.tensor.matmul(paggs[w][:], lhsT=xl[:], rhs=selfoh[h][:],
                                     start=(mm_count[w] == 0),
                                     stop=(mm_count[w] == n_mm[w] - 1))
                    mm_count[w] += 1

            # round-robin the gather tiles across the group's windows
            tl = {w: [(st, t) for st in range(2) for t in range(int(T[st, w]))]
                  for w in wins}
            pos_i = {w: 0 for w in wins}
            remaining = sum(len(v) for v in tl.values())
            while remaining:
                for w in wins:
                    if pos_i[w] >= len(tl[w]):
                        continue
                    st, t = tl[w][pos_i[w]]
                    pos_i[w] += 1
                    remaining -= 1
                    gt = int(tile_start[st, w]) + t
                    c, blk = divmod(gt, CH // TILE)
                    gx = chunk_tile(st, c)
                    oh = oh_chunk_tile(st, c)
                    nc.tensor.matmul(
                        paggs[w][:],
                        lhsT=gx[:, blk:blk + 1, :],
                        rhs=oh[:, blk, :],
                        start=(mm_count[w] == 0),
                        stop=(mm_count[w] == n_mm[w] - 1),
                    )
                    mm_count[w] += 1

            for w in wins:
                epilogue(w, paggs[w])

    nc.compile()
    return nc


def kernel(x, edge_index, W, b, alpha):
    from concourse.bass_utils import run_bass_kernel_spmd

    t0 = time.time()
    x = np.ascontiguousarray(np.asarray(x, dtype=np.float32))
    W = np.asarray(W, dtype=np.float32)
    b = np.asarray(b, dtype=np.float32)
    alpha = np.asarray(alpha, dtype=np.float32)

    per_core, T, tile_start, NT, xs = _plan(edge_index, x, W, alpha)
    fast_path = bool(np.all(b == 0.0) and np.all(alpha > 0.0))
    uniform_alpha = bool(np.all(b == 0.0) and np.all(alpha == alpha[0])
                         and alpha[0] >= 0.0)
    alpha0 = float(alpha[0])

    wt = np.ascontiguousarray(W.T.astype(np.float16))                     # [F, H]
    w1t = np.ascontiguousarray((alpha[:, None] * W).T.astype(np.float16))  # [F, H]
    dlself = np.stack([np.arange(128, dtype=np.float32),
                       np.arange(128, dtype=np.float32) + 128], axis=1)
    dlself = np.ascontiguousarray(dlself)  # [128, 2]

    t1 = time.time()
    nc = _build_program(T, tile_start, NT, fast_path, uniform_alpha, alpha0)
    t2 = time.time()

    in_maps = []
    for k in range(NC_CORES):
        d = per_core[k]
        m = {
            "xs": xs, "w_t": wt, "w1_t": w1t,
            "xloc": d["xloc"],
            "idx0": d[("idx", 0)], "idx1": d[("idx", 1)],
            "dlc0": d[("dlc", 0)], "dlc1": d[("dlc", 1)],
            "dv": d["dv"], "dvn": d["dvn"],
            "dlself": dlself,
        }
        if not fast_path:
            m["alpha_row"] = alpha.reshape(1, H).astype(np.float32)
            m["b_row"] = b.reshape(1, H).astype(np.float32)
        in_maps.append(m)

    trace = bool(int(os.environ.get("GCN_BASS_TRACE", "0")))
    res = run_bass_kernel_spmd(nc, in_maps, core_ids=list(range(NC_CORES)),
                               trace=trace)
    t3 = time.time()

    outs = np.stack([res.results[k]["out"][:ND].astype(np.float32)
                     for k in range(NC_CORES)])  # [8, 6250, H]
    out_full = outs.transpose(1, 0, 2).reshape(N, H)
    t4 = time.time()

    last_run_info.update({
        "exec_time_ns": res.exec_time_ns,
        "plan_s": t1 - t0, "build_s": t2 - t1, "run_s": t3 - t2,
        "unshard_s": t4 - t3, "fast_path": fast_path,
        "NT": NT, "trace": trace,
    })
    return out_full
